# revision 38
# baseline (speedup 1.0000x reference)
"""MultiHeadAttention on 8 TRN2 NeuronCores: DP=2 (batch) x TP=4 (heads).

Shapes (hardcoded): x [4, 2048, 1024], 16 heads x 64 dim, causal.
Per core: 2 batches, 4 heads. Device computes QKV, causal softmax
attention (with an augmented ones-column in V to produce softmax
denominators), and a partial out-projection over its 256 v-dims.
Host sums partials over the 4 TP cores (all-reduce) and adds bo.

v4: all matmul operands and the y output in bf16 (PE rate is identical
to fp32r at large N but has no N>=256 constraint, so the d3 widening
hack is gone; DMA bytes halve; rel-err ~4e-3 vs the 2e-2 budget).
DMAs fused into few large transfers (the shared HWDGE descriptor
engine serializes at ~630ns per DMA instruction); all DMAs ride the
SP/Activation HW DGE queues (gpsimd DMAs execute on the Pool engine
itself). AV matmuls deferred two score tiles so the scalar-engine exp
stays off the PE critical path. Causal masks on Pool. Out-projection
interleaved per q-chunk.
"""

import os
from contextlib import ExitStack

import numpy as np

import concourse.mybir as mybir
import concourse.tile as tile
from concourse import bacc

B, T, D = 4, 2048, 1024
H, HD = 16, 64
DP, TP = 2, 4
NB = B // DP          # batches per core
NH = H // TP          # heads per core
HV = HD + 1           # head dim + ones column
VA = NH * HV          # 260 augmented v columns
QH = NH * HD          # 256 q/k columns per core
SCALE = 1.0 / 8.0     # 1/sqrt(HD)

LAST_EXEC_NS = None
_NC = None


def build_nc():
    f32 = mybir.dt.float32
    bf = mybir.dt.bfloat16
    Exp = mybir.ActivationFunctionType.Exp
    Ln = mybir.ActivationFunctionType.Ln
    Copy = mybir.ActivationFunctionType.Copy

    nc = bacc.Bacc(trn_type="TRN2", target_bir_lowering=False, debug=False)
    xt = nc.declare_dram_parameter("xt", [NB * D, T], bf, isOutput=False)
    wq = nc.declare_dram_parameter("wq", [D, QH], bf, isOutput=False)
    wk = nc.declare_dram_parameter("wk", [D, QH], bf, isOutput=False)
    wv = nc.declare_dram_parameter("wv", [D, VA], bf, isOutput=False)
    wo = nc.declare_dram_parameter("wo", [QH, D], bf, isOutput=False)
    bqk = nc.declare_dram_parameter("bqk", [QH, 2], f32, isOutput=False)
    bv = nc.declare_dram_parameter("bv", [1, VA], bf, isOutput=False)
    # tr = upper-triangle ones: post-exp causal mask multiply on Vector
    tr = nc.declare_dram_parameter("tr", [128, 128], bf, isOutput=False)
    y = nc.declare_dram_parameter("y", [NB * T, D], bf, isOutput=True)

    with tile.TileContext(nc) as tc, ExitStack() as ctx:
        # pin the gpsimd ucode library to `proxy` (has BOTH TensorTensor
        # and PartitionBroadcast): the auto-inserted per-op library loads
        # otherwise thrash standard<->attn around every broadcast, and
        # each swap is a hidden ~7us microcode DMA the PE chain waits on
        from concourse import library_config
        nc.gpsimd.load_library(library_config.proxy)
        # likewise pin the scalar activation table to the set holding
        # BOTH Exp and Ln: the auto-inserted per-function table loads
        # otherwise alternate exp<->ln sets (~1.3us each) at every
        # head-pair normalize
        from concourse.hw_specs import get_activation_tables
        lnexp_id = list(get_activation_tables(nc.m.arch)).index(
            "natural_log_exp_and_others")
        nc.scalar.add_instruction(mybir.InstLoadActFuncSet(
            name=nc.get_next_instruction_name(),
            act_func_set_id=lnexp_id))
        cpool = ctx.enter_context(tc.tile_pool(name="const", bufs=1))
        ppool = ctx.enter_context(tc.tile_pool(name="persist", bufs=1))
        xpool = ctx.enter_context(tc.tile_pool(name="xin", bufs=2))
        epool = ctx.enter_context(tc.tile_pool(name="escores", bufs=9))
        ypool = ctx.enter_context(tc.tile_pool(name="yout", bufs=2))
        rpool = ctx.enter_context(tc.tile_pool(name="recip", bufs=2))
        psum = ctx.enter_context(tc.tile_pool(name="ps", bufs=1, space="PSUM"))

        def blk(dram, nblk):
            return dram[:, :].rearrange("(blk p) c -> p blk c", p=128)

        # ---- startup-critical loads, interleaved on the sync queue so
        # the first projection group can start ASAP: wq and x(chunk 0)
        # in 2-block pieces, alternating ----
        wqf = cpool.tile([128, 8, QH], bf, tag="wqf")
        xf0 = xpool.tile([128, 8, 512], bf, tag="x")
        wq_r = blk(wq, 8)
        x0_r = xt[0:D, 0:512].rearrange("(blk p) c -> p blk c", p=128)
        # wq and x0 interleaved on the sync HWDGE queue; the PE warmup
        # stream (emit_boot) covers the serialized issue latency, and
        # avoiding SWDGE keeps gpsimd drain ops out of the kernel
        xp = [(0, 1), (1, 3), (3, 5), (5, 8)]
        for g in range(4):
            nc.sync.dma_start(wqf[:, 2 * g:2 * g + 2, :],
                              wq_r[:, 2 * g:2 * g + 2, :])
            lo, hi = xp[g]
            nc.sync.dma_start(xf0[:, lo:hi, :], x0_r[:, lo:hi, :])
        # remaining consts on the scalar queue (needed later than wq/x0)
        bqk_t = cpool.tile([128, 2, 2], f32, tag="bqk")
        nc.scalar.dma_start(bqk_t[:], blk(bqk, 2))
        wkf = cpool.tile([128, 8, QH], bf, tag="wkf")
        wk_r = blk(wk, 8)
        for g in range(2):
            nc.scalar.dma_start(wkf[:, 4 * g:4 * g + 4, :],
                                wk_r[:, 4 * g:4 * g + 4, :])
        wvf = cpool.tile([128, 8, VA], bf, tag="wvf")
        wv_r = blk(wv, 8)
        for g in range(2):
            nc.scalar.dma_start(wvf[:, 4 * g:4 * g + 4, :],
                                wv_r[:, 4 * g:4 * g + 4, :])
        bv_l = cpool.tile([1, VA], bf, tag="bv")
        nc.scalar.dma_start(bv_l[:], bv[:, :])
        tr_t = cpool.tile([128, 128], bf, tag="tr")
        nc.scalar.dma_start(tr_t[:], tr[:, :])

        def load_late_consts():
            # bias row broadcast across partitions for the V copy-add
            bvb_l = cpool.tile([128, VA], bf, tag="bvb")
            nc.gpsimd.partition_broadcast(bvb_l[:], bv_l[:], channels=128)
            wo_l = cpool.tile([128, 2, D], bf, tag="wof")
            nc.scalar.dma_start(wo_l[:], blk(wo, 2))
            return bvb_l, wo_l

        # persistent tiles: qt/ot single-buffered (chunk lifetimes are
        # disjoint across batches in the pipelined schedule); kt/va
        # double-buffered by batch parity (live across a whole batch).
        # zero stationary for HAM warm-up matmul streams (boot + tail)
        zt = cpool.tile([128, 512], bf, tag="warm")
        nc.vector.memset(zt[:], 0)

        qt_t = [ppool.tile([128, T], bf, tag=f"qt{p}", name=f"qt{p}")
                for p in range(2)]
        ot_t = [ppool.tile([128, T], bf, tag=f"ot{p}", name=f"ot{p}")
                for p in range(2)]
        kt_t = [[ppool.tile([128, T], bf, tag=f"kt{par}{p}",
                            name=f"kt{par}{p}")
                 for p in range(2)] for par in range(2)]
        va_t = [[ppool.tile([128, VA], bf, tag=f"va{par}{i}",
                            name=f"va{par}{i}")
                 for i in range(16)] for par in range(2)]

        def emit_loads(ci):
            b, j = divmod(ci, 4)
            xf = xpool.tile([128, 8, 512], bf, tag="x")
            src = xt[b * D:(b + 1) * D,
                     512 * j:512 * (j + 1)].rearrange(
                         "(blk p) c -> p blk c", p=128)
            nc.sync.dma_start(xf[:], src)
            return xf

        def make_closures(ci, xf):
            """Phase-A psum groups for chunk ci as deferred emitters."""
            b, j = divmod(ci, 4)
            par = b % 2
            cl = []

            def proj(p, w_f, dst, b_ap):
                def f():
                    pp = psum.tile([128, 512], f32, tag="a", bufs=2)
                    for dc in range(8):
                        nc.tensor.matmul(
                            pp[:],
                            w_f[:, dc, 128 * p:128 * (p + 1)],
                            xf[:, dc, :],
                            start=(dc == 0), stop=(dc == 7))
                    nc.vector.tensor_scalar_add(
                        dst[:, 512 * j:512 * (j + 1)], pp[:], b_ap)
                return f

            qcl = []
            for p in range(2):
                qcl.append(proj(p, wqf, qt_t[p], bqk_t[:, p, 0:1]))
            for p in range(2):
                cl.append(proj(p, wkf, kt_t[par][p], bqk_t[:, p, 1:2]))

            def vproj(tt):
                def f():
                    pv = psum.tile([128, 512], f32, tag="a", bufs=2)
                    for dc in range(8):
                        nc.tensor.matmul(
                            pv[:, 0:VA],
                            xf[:, dc, 128 * tt:128 * (tt + 1)],
                            wvf[:, dc, :],
                            start=(dc == 0), stop=(dc == 7))
                    nc.vector.tensor_add(
                        va_t[par][4 * j + tt][:], pv[:, 0:VA], bvb_t[:])
                return f

            for tt in range(4):
                cl.append(vproj(tt))
            # (q-closures, kv-closures): q must precede the chunk's
            # unit; k/v are only read from score tile 4j on, so they
            # may pace inside the chunk's own unit
            return qcl, cl

        def emit_unit(ci, closures, stride_add=0, split_norm=False):
            """B (attention) + C (out-proj) for chunk ci, interleaving
            the next chunk's phase-A groups between score groups."""
            b, j = divmod(ci, 4)
            par = b % 2
            ni = 4 * j + 4
            # pace closures evenly across the whole h-loop (they only
            # must complete by the end of this unit); bunching them
            # early leaves the last tiles ACT-throttled with no fill;
            # stride_add shifts some fill past the flush (tail units)
            stride = max(1, (2 * ni + stride_add) //
                         max(1, len(closures)))
            cnt = 0
            cidx = 0
            # heads processed in quadrant-packed pairs: the two heads
            # sharing one kt/qt tile issue their K=64 score matmuls into
            # distinct PE row quadrants (tile_position row 0 vs 64);
            # h0 packs at s-cols 0:w, h1 at 512:512+w of one [128,1024]
            # score tile
            for hp in range(2):
                h0, h1 = 2 * hp, 2 * hp + 1
                ov0 = psum.tile([128, 512], f32, tag="ov", bufs=2)
                ov1 = psum.tile([128, 512], f32, tag="ov", bufs=2)

                def emit_av(i, st, w, c1, ep):
                    for (h, ov, c) in ((h0, ov0, 0), (h1, ov1, c1)):
                        nc.tensor.matmul(
                            ov[0:HV, st:512],
                            va_t[par][i][:, HV * h:HV * (h + 1)],
                            ep[:, c:c + w],
                            start=(i == 0), stop=(i == ni - 1))

                pend = []
                for i in range(ni):
                    di = i - 4 * j
                    if di <= 0:
                        st, w = 0, 512
                    else:
                        st, w = 128 * di, 512 - 128 * di
                    sc = psum.tile([128, 1024], f32, tag="s", bufs=2)
                    et = epool.tile([128, 1024], bf, tag="e")
                    # h1 stays at col base 512: the pair's concurrent
                    # quadrant matmuls must write DIFFERENT PSUM banks
                    # (adjacent packing same-bank hangs the device)
                    c1 = 512
                    for (hr, c) in ((0, 0), (1, c1)):
                        nc.tensor.matmul(
                            sc[:, c:c + w],
                            kt_t[par][hp][64 * hr:64 * hr + 64,
                                          128 * i:128 * (i + 1)],
                            qt_t[hp][64 * hr:64 * hr + 64,
                                     512 * j + st:512 * (j + 1)],
                            start=True, stop=True,
                            tile_position=(64 * hr, 0))
                    if c1 == w:
                        nc.scalar.activation(
                            et[:, 0:2 * w], sc[:, 0:2 * w],
                            Exp, scale=SCALE)
                    elif w == 512:
                        nc.scalar.activation(
                            et[:, 0:1024], sc[:, 0:1024],
                            Exp, scale=SCALE)
                    else:
                        nc.scalar.activation(
                            et[:, 0:w], sc[:, 0:w], Exp, scale=SCALE)
                        nc.scalar.activation(
                            et[:, 512:512 + w], sc[:, 512:512 + w],
                            Exp, scale=SCALE)
                    if 0 <= di <= 3:
                        # masks on Vector: gpsimd ops carry ~1us of
                        # semaphore latency each, which the paired AV
                        # matmuls end up waiting on
                        nc.vector.tensor_mul(
                            et[:, 0:128], et[:, 0:128], tr_t[:])
                        nc.vector.tensor_mul(
                            et[:, c1:c1 + 128], et[:, c1:c1 + 128],
                            tr_t[:])
                    # AV deferred seven tiles: flush slack keeps the
                    # mask/exp chain off the PE critical path
                    if len(pend) >= 7:
                        emit_av(*pend.pop(0))
                    pend.append((i, st, w, c1, et))
                    cnt += 1
                    if cidx < len(closures) and cnt % stride == 0:
                        closures[cidx]()
                        cidx += 1
                for p_ in pend:
                    emit_av(*p_)
                # 1/denom as exp(-ln(x)) on the Scalar engine: two
                # table activations ~0.6us each (rel err ~1e-5) vs
                # 3.3us for the column-paced DVE reciprocal, and the
                # hr0/hr1 chains pipeline instead of serializing on
                # Vector — this chain gates the ov PSUM rotation the
                # next head-pair's AV matmuls wait on.
                # split_norm (final unit): normalize in 256-col halves
                # ordered half0(hr0,hr1) then half1, so the final
                # out-projection's first q-blocks start ~2us earlier
                halves = ((0, 256), (256, 512)) if split_norm \
                    else ((0, 512),)
                for (lo, hi) in halves:
                    for (hr, ov) in ((0, ov0), (1, ov1)):
                        w_ = hi - lo
                        lt = rpool.tile([1, 512], f32, tag="l")
                        rt = rpool.tile([1, 512], f32, tag="r")
                        nc.scalar.activation(
                            lt[:, 0:w_], ov[64:65, lo:hi], Ln)
                        nc.scalar.activation(
                            rt[:, 0:w_], lt[:, 0:w_], Exp, scale=-1.0)
                        bc = rpool.tile([64, 512], f32, tag="bc")
                        nc.gpsimd.partition_broadcast(
                            bc[:, 0:w_], rt[:, 0:w_], channels=64)
                        nc.vector.tensor_mul(
                            ot_t[hp][64 * hr:64 * hr + 64,
                                     512 * j + lo:512 * j + hi],
                            ov[0:64, lo:hi], bc[:, 0:w_])

            while cidx < len(closures):
                closures[cidx]()
                cidx += 1

        def make_phase_c(ci, fine=False):
            """Out-projection closures for chunk ci, deferred into the
            next unit's h-loop as PE fill work. fine=True issues one y
            DMA per q-block (shorter drain for the final chunk)."""
            b, j = divmod(ci, 4)
            cl = []
            for qg in range(2):
                yt = ypool.tile([128, 2, D], bf, tag="y")
                for qq in range(2 * qg, 2 * qg + 2):
                    for do_ in range(2):
                        first = fine and qg == 0 and qq == 0 and do_ == 0
                        def f(yt=yt, qg=qg, qq=qq, do_=do_, first=first):
                            q = 4 * j + qq
                            yp = psum.tile([128, 512], f32, tag="a",
                                           bufs=2)
                            if first:
                                # dep-free zero-weight MMs bridge the
                                # final normalize-chain PE idle (~3.2us,
                                # right at the HAM MID window) so the
                                # tail out-projection runs at 2.4GHz
                                for r in range(6):
                                    nc.tensor.matmul(
                                        yp[:], zt[:, 0:128], zt[:],
                                        start=(r == 0), stop=False)
                            for kc in range(2):
                                nc.tensor.matmul(
                                    yp[:],
                                    ot_t[kc][:, 128 * q:128 * (q + 1)],
                                    wo_t[:, kc,
                                         512 * do_:512 * (do_ + 1)],
                                    start=(kc == 0 and not first),
                                    stop=(kc == 1))
                            if fine:
                                # last chunk: Scalar is idle (exp done)
                                # and Vector CASTs gate the a-pool
                                # rotation the tail matmuls wait on
                                nc.scalar.activation(
                                    yt[:, qq - 2 * qg,
                                       512 * do_:512 * (do_ + 1)],
                                    yp[:], Copy)
                            else:
                                nc.vector.tensor_copy(
                                    yt[:, qq - 2 * qg,
                                       512 * do_:512 * (do_ + 1)], yp[:])
                            if do_ == 1 and fine:
                                nc.sync.dma_start(
                                    y[b * T + 128 * q:
                                      b * T + 128 * (q + 1), :],
                                    yt[:, qq - 2 * qg, :])
                            elif do_ == 1 and qq == 2 * qg + 1:
                                q0 = 4 * j + 2 * qg
                                dst = y[b * T + 128 * q0:
                                        b * T + 128 * (q0 + 2),
                                        :].rearrange(
                                            "(blk p) c -> p blk c", p=128)
                                nc.sync.dma_start(dst, yt[:])
                        cl.append(f)
            return cl

        def emit_boot(xf):
            """Chunk 0's A-phase with the four q/k projection groups
            interleaved at the dc level (the q and k pairs live in the
            halves of two score-PSUM tiles, idle at boot), so PE
            progress tracks the x-block DMA feed instead of serializing
            group-by-group behind it."""
            sq = psum.tile([128, 1024], f32, tag="s", bufs=2)
            sk = psum.tile([128, 1024], f32, tag="s", bufs=2)
            # HAM warm-up: the PE clock boots throttled (K=4/8, 1.2GHz)
            # and un-throttles only after ~3.4us of sustained activity.
            # While the wq/x0 DMAs land, stream ~4us of zero-weight
            # matmuls into the sq accumulation group (0 contribution;
            # the real q-proj below joins the group with start=False)
            # ~26 x 512-col cold MMs ~ 11us: bridges the wq/x0 DMA
            # window (~12us) so HAM doesn't re-throttle before the
            # first real matmul
            for r in range(56):
                nc.tensor.matmul(
                    sq[:, 512 * (r % 2):512 * (r % 2 + 1)],
                    zt[:, 0:128], zt[:],
                    start=(r < 2), stop=False)
            for dc in range(8):
                for p in range(2):
                    nc.tensor.matmul(
                        sq[:, 512 * p:512 * (p + 1)],
                        wqf[:, dc, 128 * p:128 * (p + 1)],
                        xf[:, dc, :], start=False, stop=(dc == 7))
                    nc.tensor.matmul(
                        sk[:, 512 * p:512 * (p + 1)],
                        wkf[:, dc, 128 * p:128 * (p + 1)],
                        xf[:, dc, :], start=(dc == 0), stop=(dc == 7))
            for p in range(2):
                nc.vector.tensor_scalar_add(
                    qt_t[p][:, 0:512], sq[:, 512 * p:512 * (p + 1)],
                    bqk_t[:, p, 0:1])
                nc.vector.tensor_scalar_add(
                    kt_t[0][p][:, 0:512], sk[:, 512 * p:512 * (p + 1)],
                    bqk_t[:, p, 1:2])

            def ve(tt):
                def e(pv, dc):
                    nc.tensor.matmul(
                        pv[:, 0:VA], xf[:, dc, 128 * tt:128 * (tt + 1)],
                        wvf[:, dc, :], start=(dc == 0), stop=(dc == 7))
                return e

            def vfin(tt):
                def f(pv):
                    nc.vector.tensor_add(
                        va_t[0][tt][:], pv[:, 0:VA], bvb_t[:])
                return f

            for t0_ in (0, 2):
                pa = psum.tile([128, 512], f32, tag="a", bufs=2)
                pb = psum.tile([128, 512], f32, tag="a", bufs=2)
                for dc in range(8):
                    ve(t0_)(pa, dc)
                    ve(t0_ + 1)(pb, dc)
                vfin(t0_)(pa)
                vfin(t0_ + 1)(pb)

        def mix(a, b):
            out = []
            la, lb = list(a), list(b)
            while la or lb:
                if la:
                    out.append(la.pop(0))
                if lb:
                    out.append(lb.pop(0))
            return out

        # ---- pipelined schedule over 8 chunks (2 batches x 4 j) ----
        # Fill assignment per unit, balanced to each unit's absorbable
        # slack (big-j units run ACT-throttled without PE fill; unit 7
        # gets its own chunk's k/v projections, legal since those are
        # only read from score tile 4j on, plus two C sets):
        #   unit0: A(1) | unit1..4: C(ci-1)+A(ci+1) | unit5: A(6)
        #   unit6: C(4)+C(5)+A(7).q | unit7: A(7).kv+C(6) | end: C(7)
        bvb_t, wo_t = load_late_consts()
        emit_boot(xf0)
        xs = emit_loads(1)
        a_next = make_closures(1, xs)     # (q, kv) for chunk 1
        a7 = None
        csets = {}
        for ci in range(8):
            if ci + 2 <= 7:
                xs = emit_loads(ci + 2)
            if ci == 0:
                fill = a_next[0] + a_next[1]
            elif ci <= 4:
                # lead with A (projection) closures: the C(ci-1)
                # out-proj matmuls read ot, which waits on the previous
                # unit's normalize chain — putting one first parks it at
                # the PE FIFO head and stalls the whole unit behind it
                an = a_next[0] + a_next[1]
                fill = an[:4] + mix(csets.pop(ci - 1), an[4:])
            elif ci == 5:
                fill = a_next[0] + a_next[1]
            elif ci == 6:
                fill = mix(csets.pop(4) + csets.pop(5), a7[0])
            else:
                # kv first (deadline: AV reads va tiles 12-15 from the
                # hp0 flush, ~count 16), then two no-op slots to shift
                # C(6) into the back half where fill is scarce; hold 3
                # C(6) groups back for after the normalize section
                c6 = csets.pop(6)
                fill = a7[1] + [lambda: None] * 2 + c6[:3]
                held = c6[3:]
            emit_unit(ci, fill, split_norm=(ci == 7))
            if ci == 7:
                for f in held:
                    f()
            csets[ci] = make_phase_c(ci, fine=(ci == 7))
            if ci + 2 <= 7:
                nxt = make_closures(ci + 2, xs)
                if ci + 2 == 7:
                    a7 = nxt
                else:
                    a_next = nxt
        for f in csets.pop(7):
            f()

    nc.compile()
    return nc


def make_in_maps(inputs):
    import ml_dtypes
    bf = ml_dtypes.bfloat16
    x = inputs["x"].astype(np.float32)
    Wq, Wk, Wv, Wo = (inputs[k].astype(np.float32)
                      for k in ("Wq", "Wk", "Wv", "Wo"))
    bq, bk, bv = (inputs[k].astype(np.float32) for k in ("bq", "bk", "bv"))
    in_maps = []
    for c in range(8):
        dp, tp = divmod(c, TP)
        xt = np.ascontiguousarray(np.concatenate(
            [x[NB * dp + bb].T for bb in range(NB)], axis=0)).astype(bf)
        wq_c = np.ascontiguousarray(Wq[:, tp * QH:(tp + 1) * QH]).astype(bf)
        wk_c = np.ascontiguousarray(Wk[:, tp * QH:(tp + 1) * QH]).astype(bf)
        wv_c = np.zeros((D, VA), np.float32)
        bv_c = np.zeros((1, VA), np.float32)
        for hh in range(NH):
            g = tp * NH + hh
            wv_c[:, HV * hh:HV * hh + HD] = Wv[:, g * HD:(g + 1) * HD]
            bv_c[0, HV * hh:HV * hh + HD] = bv[g * HD:(g + 1) * HD]
            bv_c[0, HV * hh + HD] = 1.0
        wo_c = np.ascontiguousarray(Wo[tp * QH:(tp + 1) * QH, :]).astype(bf)
        bqk_c = np.ascontiguousarray(np.stack(
            [bq[tp * QH:(tp + 1) * QH], bk[tp * QH:(tp + 1) * QH]], axis=1))
        in_maps.append({
            "xt": xt, "wq": wq_c, "wk": wk_c, "wv": wv_c.astype(bf),
            "wo": wo_c, "bqk": bqk_c, "bv": bv_c.astype(bf),
            "tr": np.triu(np.ones((128, 128), np.float32)).astype(bf),
        })
    return in_maps


def kernel(**inputs):
    global LAST_EXEC_NS, _NC
    from concourse.bass_utils import run_bass_kernel_spmd

    if _NC is None:
        _NC = build_nc()
    in_maps = make_in_maps(inputs)
    res = run_bass_kernel_spmd(_NC, in_maps, core_ids=list(range(8)))

    bo = inputs["bo"].astype(np.float64)
    y_full = np.zeros((B, T, D), np.float64)
    for c in range(8):
        dp, tp = divmod(c, TP)
        yc = np.asarray(res.results[c]["y"]).astype(np.float64)
        for bb in range(NB):
            y_full[NB * dp + bb] += yc[bb * T:(bb + 1) * T, :]
    y_full += bo
    return y_full.astype(np.float32)



# revision 39
# speedup vs baseline: 1.1945x; 1.1945x over previous
"""MultiHeadAttention on 8 TRN2 NeuronCores: DP=2 (batch) x TP=4 (heads).

Shapes (hardcoded): x [4, 2048, 1024], 16 heads x 64 dim, causal.
Per core: 2 batches, 4 heads. Device computes QKV, causal softmax
attention (with an augmented ones-column in V to produce softmax
denominators), and a partial out-projection over its 256 v-dims.
Host sums partials over the 4 TP cores (all-reduce) and adds bo.

v4: all matmul operands and the y output in bf16; DMAs fused into few
large transfers on the SP/Activation HWDGE queues; AV matmuls
deferred so the scalar-engine exp stays off the PE critical path;
out-projection interleaved per q-chunk as PE fill work.

v17 (this version), found via perfetto/NTFF trace analysis:
- gpsimd ucode library pinned to `proxy` (TensorTensor +
  PartitionBroadcast in one library): the auto-inserted per-op loads
  thrashed standard<->attn at every head-pair boundary, a hidden
  ~7us microcode DMA the PE dependency chain waited on.
- softmax 1/denom as exp(-ln(x)) on the Scalar engine (two table
  activations, ~1e-5 rel err) instead of the 3.3us column-paced DVE
  reciprocal; activation table pinned to the set holding BOTH Exp
  and Ln to kill per-boundary table reloads.
- causal masks on Vector (gpsimd ops carry ~1us semaphore latency
  the paired AV matmuls waited on).
- A-closures lead the unit fills: a C(ci-1) out-proj matmul emitted
  first parks at the PE queue head waiting on the previous unit's
  normalize chain and stalls the whole unit.
- HAM warm-up: zero-weight matmul streams at boot (bridging the
  first DMA window) and into the tail out-projection, so the PE
  clock-gate (K=4/8 at 1.2GHz when idle >3.4us) stays at 2.4GHz.
- final unit's normalize split into 256-col halves so the last
  chunk's out-projection starts ~2us earlier.
"""

import os
from contextlib import ExitStack

import numpy as np

import concourse.mybir as mybir
import concourse.tile as tile
from concourse import bacc

B, T, D = 4, 2048, 1024
H, HD = 16, 64
DP, TP = 2, 4
NB = B // DP          # batches per core
NH = H // TP          # heads per core
HV = HD + 1           # head dim + ones column
VA = NH * HV          # 260 augmented v columns
QH = NH * HD          # 256 q/k columns per core
SCALE = 1.0 / 8.0     # 1/sqrt(HD)

LAST_EXEC_NS = None
_NC = None


def build_nc():
    f32 = mybir.dt.float32
    bf = mybir.dt.bfloat16
    Exp = mybir.ActivationFunctionType.Exp
    Ln = mybir.ActivationFunctionType.Ln
    Copy = mybir.ActivationFunctionType.Copy

    nc = bacc.Bacc(trn_type="TRN2", target_bir_lowering=False, debug=False)
    xt = nc.declare_dram_parameter("xt", [NB * D, T], bf, isOutput=False)
    wq = nc.declare_dram_parameter("wq", [D, QH], bf, isOutput=False)
    wk = nc.declare_dram_parameter("wk", [D, QH], bf, isOutput=False)
    wv = nc.declare_dram_parameter("wv", [D, VA], bf, isOutput=False)
    wo = nc.declare_dram_parameter("wo", [QH, D], bf, isOutput=False)
    bqk = nc.declare_dram_parameter("bqk", [QH, 2], f32, isOutput=False)
    bv = nc.declare_dram_parameter("bv", [1, VA], bf, isOutput=False)
    # tr = upper-triangle ones: post-exp causal mask multiply on Vector
    tr = nc.declare_dram_parameter("tr", [128, 128], bf, isOutput=False)
    y = nc.declare_dram_parameter("y", [NB * T, D], bf, isOutput=True)

    with tile.TileContext(nc) as tc, ExitStack() as ctx:
        # pin the gpsimd ucode library to `proxy` (has BOTH TensorTensor
        # and PartitionBroadcast): the auto-inserted per-op library loads
        # otherwise thrash standard<->attn around every broadcast, and
        # each swap is a hidden ~7us microcode DMA the PE chain waits on
        from concourse import library_config
        nc.gpsimd.load_library(library_config.proxy)
        # likewise pin the scalar activation table to the set holding
        # BOTH Exp and Ln: the auto-inserted per-function table loads
        # otherwise alternate exp<->ln sets (~1.3us each) at every
        # head-pair normalize
        from concourse.hw_specs import get_activation_tables
        lnexp_id = list(get_activation_tables(nc.m.arch)).index(
            "natural_log_exp_and_others")
        nc.scalar.add_instruction(mybir.InstLoadActFuncSet(
            name=nc.get_next_instruction_name(),
            act_func_set_id=lnexp_id))
        cpool = ctx.enter_context(tc.tile_pool(name="const", bufs=1))
        ppool = ctx.enter_context(tc.tile_pool(name="persist", bufs=1))
        xpool = ctx.enter_context(tc.tile_pool(name="xin", bufs=2))
        epool = ctx.enter_context(tc.tile_pool(name="escores", bufs=9))
        ypool = ctx.enter_context(tc.tile_pool(name="yout", bufs=2))
        rpool = ctx.enter_context(tc.tile_pool(name="recip", bufs=2))
        psum = ctx.enter_context(tc.tile_pool(name="ps", bufs=1, space="PSUM"))

        def blk(dram, nblk):
            return dram[:, :].rearrange("(blk p) c -> p blk c", p=128)

        # ---- startup-critical loads, interleaved on the sync queue so
        # the first projection group can start ASAP: wq and x(chunk 0)
        # in 2-block pieces, alternating ----
        wqf = cpool.tile([128, 8, QH], bf, tag="wqf")
        xf0 = xpool.tile([128, 8, 512], bf, tag="x")
        wq_r = blk(wq, 8)
        x0_r = xt[0:D, 0:512].rearrange("(blk p) c -> p blk c", p=128)
        # wq and x0 interleaved on the sync HWDGE queue; the PE warmup
        # stream (emit_boot) covers the serialized issue latency, and
        # avoiding SWDGE keeps gpsimd drain ops out of the kernel
        xp = [(0, 1), (1, 3), (3, 5), (5, 8)]
        for g in range(4):
            nc.sync.dma_start(wqf[:, 2 * g:2 * g + 2, :],
                              wq_r[:, 2 * g:2 * g + 2, :])
            lo, hi = xp[g]
            nc.sync.dma_start(xf0[:, lo:hi, :], x0_r[:, lo:hi, :])
        # remaining consts on the scalar queue (needed later than wq/x0)
        bqk_t = cpool.tile([128, 2, 2], f32, tag="bqk")
        nc.scalar.dma_start(bqk_t[:], blk(bqk, 2))
        wkf = cpool.tile([128, 8, QH], bf, tag="wkf")
        wk_r = blk(wk, 8)
        for g in range(2):
            nc.scalar.dma_start(wkf[:, 4 * g:4 * g + 4, :],
                                wk_r[:, 4 * g:4 * g + 4, :])
        wvf = cpool.tile([128, 8, VA], bf, tag="wvf")
        wv_r = blk(wv, 8)
        for g in range(2):
            nc.scalar.dma_start(wvf[:, 4 * g:4 * g + 4, :],
                                wv_r[:, 4 * g:4 * g + 4, :])
        bv_l = cpool.tile([1, VA], bf, tag="bv")
        nc.scalar.dma_start(bv_l[:], bv[:, :])
        tr_t = cpool.tile([128, 128], bf, tag="tr")
        nc.scalar.dma_start(tr_t[:], tr[:, :])

        def load_late_consts():
            # bias row broadcast across partitions for the V copy-add
            bvb_l = cpool.tile([128, VA], bf, tag="bvb")
            nc.gpsimd.partition_broadcast(bvb_l[:], bv_l[:], channels=128)
            wo_l = cpool.tile([128, 2, D], bf, tag="wof")
            nc.scalar.dma_start(wo_l[:], blk(wo, 2))
            return bvb_l, wo_l

        # persistent tiles: qt/ot single-buffered (chunk lifetimes are
        # disjoint across batches in the pipelined schedule); kt/va
        # double-buffered by batch parity (live across a whole batch).
        # zero stationary for HAM warm-up matmul streams (boot + tail)
        zt = cpool.tile([128, 512], bf, tag="warm")
        nc.vector.memset(zt[:], 0)

        qt_t = [ppool.tile([128, T], bf, tag=f"qt{p}", name=f"qt{p}")
                for p in range(2)]
        ot_t = [ppool.tile([128, T], bf, tag=f"ot{p}", name=f"ot{p}")
                for p in range(2)]
        kt_t = [[ppool.tile([128, T], bf, tag=f"kt{par}{p}",
                            name=f"kt{par}{p}")
                 for p in range(2)] for par in range(2)]
        va_t = [[ppool.tile([128, VA], bf, tag=f"va{par}{i}",
                            name=f"va{par}{i}")
                 for i in range(16)] for par in range(2)]

        def emit_loads(ci):
            b, j = divmod(ci, 4)
            xf = xpool.tile([128, 8, 512], bf, tag="x")
            src = xt[b * D:(b + 1) * D,
                     512 * j:512 * (j + 1)].rearrange(
                         "(blk p) c -> p blk c", p=128)
            nc.sync.dma_start(xf[:], src)
            return xf

        def make_closures(ci, xf):
            """Phase-A psum groups for chunk ci as deferred emitters."""
            b, j = divmod(ci, 4)
            par = b % 2
            cl = []

            def proj(p, w_f, dst, b_ap):
                def f():
                    pp = psum.tile([128, 512], f32, tag="a", bufs=2)
                    for dc in range(8):
                        nc.tensor.matmul(
                            pp[:],
                            w_f[:, dc, 128 * p:128 * (p + 1)],
                            xf[:, dc, :],
                            start=(dc == 0), stop=(dc == 7))
                    nc.vector.tensor_scalar_add(
                        dst[:, 512 * j:512 * (j + 1)], pp[:], b_ap)
                return f

            qcl = []
            for p in range(2):
                qcl.append(proj(p, wqf, qt_t[p], bqk_t[:, p, 0:1]))
            for p in range(2):
                cl.append(proj(p, wkf, kt_t[par][p], bqk_t[:, p, 1:2]))

            def vproj(tt):
                def f():
                    pv = psum.tile([128, 512], f32, tag="a", bufs=2)
                    for dc in range(8):
                        nc.tensor.matmul(
                            pv[:, 0:VA],
                            xf[:, dc, 128 * tt:128 * (tt + 1)],
                            wvf[:, dc, :],
                            start=(dc == 0), stop=(dc == 7))
                    nc.vector.tensor_add(
                        va_t[par][4 * j + tt][:], pv[:, 0:VA], bvb_t[:])
                return f

            for tt in range(4):
                cl.append(vproj(tt))
            # (q-closures, kv-closures): q must precede the chunk's
            # unit; k/v are only read from score tile 4j on, so they
            # may pace inside the chunk's own unit
            return qcl, cl

        def emit_unit(ci, closures, stride_add=0, split_norm=False):
            """B (attention) + C (out-proj) for chunk ci, interleaving
            the next chunk's phase-A groups between score groups."""
            b, j = divmod(ci, 4)
            par = b % 2
            ni = 4 * j + 4
            # pace closures evenly across the whole h-loop (they only
            # must complete by the end of this unit); bunching them
            # early leaves the last tiles ACT-throttled with no fill;
            # stride_add shifts some fill past the flush (tail units)
            stride = max(1, (2 * ni + stride_add) //
                         max(1, len(closures)))
            cnt = 0
            cidx = 0
            # heads processed in quadrant-packed pairs: the two heads
            # sharing one kt/qt tile issue their K=64 score matmuls into
            # distinct PE row quadrants (tile_position row 0 vs 64);
            # h0 packs at s-cols 0:w, h1 at 512:512+w of one [128,1024]
            # score tile
            for hp in range(2):
                h0, h1 = 2 * hp, 2 * hp + 1
                ov0 = psum.tile([128, 512], f32, tag="ov", bufs=2)
                ov1 = psum.tile([128, 512], f32, tag="ov", bufs=2)

                def emit_av(i, st, w, c1, ep):
                    for (h, ov, c) in ((h0, ov0, 0), (h1, ov1, c1)):
                        nc.tensor.matmul(
                            ov[0:HV, st:512],
                            va_t[par][i][:, HV * h:HV * (h + 1)],
                            ep[:, c:c + w],
                            start=(i == 0), stop=(i == ni - 1))

                pend = []
                for i in range(ni):
                    di = i - 4 * j
                    if di <= 0:
                        st, w = 0, 512
                    else:
                        st, w = 128 * di, 512 - 128 * di
                    sc = psum.tile([128, 1024], f32, tag="s", bufs=2)
                    et = epool.tile([128, 1024], bf, tag="e")
                    # h1 stays at col base 512: the pair's concurrent
                    # quadrant matmuls must write DIFFERENT PSUM banks
                    # (adjacent packing same-bank hangs the device)
                    c1 = 512
                    for (hr, c) in ((0, 0), (1, c1)):
                        nc.tensor.matmul(
                            sc[:, c:c + w],
                            kt_t[par][hp][64 * hr:64 * hr + 64,
                                          128 * i:128 * (i + 1)],
                            qt_t[hp][64 * hr:64 * hr + 64,
                                     512 * j + st:512 * (j + 1)],
                            start=True, stop=True,
                            tile_position=(64 * hr, 0))
                    if c1 == w:
                        nc.scalar.activation(
                            et[:, 0:2 * w], sc[:, 0:2 * w],
                            Exp, scale=SCALE)
                    elif w == 512:
                        nc.scalar.activation(
                            et[:, 0:1024], sc[:, 0:1024],
                            Exp, scale=SCALE)
                    else:
                        nc.scalar.activation(
                            et[:, 0:w], sc[:, 0:w], Exp, scale=SCALE)
                        nc.scalar.activation(
                            et[:, 512:512 + w], sc[:, 512:512 + w],
                            Exp, scale=SCALE)
                    if 0 <= di <= 3:
                        # masks on Vector: gpsimd ops carry ~1us of
                        # semaphore latency each, which the paired AV
                        # matmuls end up waiting on
                        nc.vector.tensor_mul(
                            et[:, 0:128], et[:, 0:128], tr_t[:])
                        nc.vector.tensor_mul(
                            et[:, c1:c1 + 128], et[:, c1:c1 + 128],
                            tr_t[:])
                    # AV deferred seven tiles: flush slack keeps the
                    # mask/exp chain off the PE critical path
                    if len(pend) >= 7:
                        emit_av(*pend.pop(0))
                    pend.append((i, st, w, c1, et))
                    cnt += 1
                    if cidx < len(closures) and cnt % stride == 0:
                        closures[cidx]()
                        cidx += 1
                for p_ in pend:
                    emit_av(*p_)
                # 1/denom as exp(-ln(x)) on the Scalar engine: two
                # table activations ~0.6us each (rel err ~1e-5) vs
                # 3.3us for the column-paced DVE reciprocal, and the
                # hr0/hr1 chains pipeline instead of serializing on
                # Vector — this chain gates the ov PSUM rotation the
                # next head-pair's AV matmuls wait on.
                # split_norm (final unit): normalize in 256-col halves
                # ordered half0(hr0,hr1) then half1, so the final
                # out-projection's first q-blocks start ~2us earlier
                halves = ((0, 256), (256, 512)) if split_norm \
                    else ((0, 512),)
                for (lo, hi) in halves:
                    for (hr, ov) in ((0, ov0), (1, ov1)):
                        w_ = hi - lo
                        lt = rpool.tile([1, 512], f32, tag="l")
                        rt = rpool.tile([1, 512], f32, tag="r")
                        nc.scalar.activation(
                            lt[:, 0:w_], ov[64:65, lo:hi], Ln)
                        nc.scalar.activation(
                            rt[:, 0:w_], lt[:, 0:w_], Exp, scale=-1.0)
                        bc = rpool.tile([64, 512], f32, tag="bc")
                        nc.gpsimd.partition_broadcast(
                            bc[:, 0:w_], rt[:, 0:w_], channels=64)
                        nc.vector.tensor_mul(
                            ot_t[hp][64 * hr:64 * hr + 64,
                                     512 * j + lo:512 * j + hi],
                            ov[0:64, lo:hi], bc[:, 0:w_])

            while cidx < len(closures):
                closures[cidx]()
                cidx += 1

        def make_phase_c(ci, fine=False):
            """Out-projection closures for chunk ci, deferred into the
            next unit's h-loop as PE fill work. fine=True issues one y
            DMA per q-block (shorter drain for the final chunk)."""
            b, j = divmod(ci, 4)
            cl = []
            for qg in range(2):
                yt = ypool.tile([128, 2, D], bf, tag="y")
                for qq in range(2 * qg, 2 * qg + 2):
                    for do_ in range(2):
                        first = fine and qg == 0 and qq == 0 and do_ == 0
                        def f(yt=yt, qg=qg, qq=qq, do_=do_, first=first):
                            q = 4 * j + qq
                            yp = psum.tile([128, 512], f32, tag="a",
                                           bufs=2)
                            if first:
                                # dep-free zero-weight MMs bridge the
                                # final normalize-chain PE idle (~3.2us,
                                # right at the HAM MID window) so the
                                # tail out-projection runs at 2.4GHz
                                for r in range(6):
                                    nc.tensor.matmul(
                                        yp[:], zt[:, 0:128], zt[:],
                                        start=(r == 0), stop=False)
                            for kc in range(2):
                                nc.tensor.matmul(
                                    yp[:],
                                    ot_t[kc][:, 128 * q:128 * (q + 1)],
                                    wo_t[:, kc,
                                         512 * do_:512 * (do_ + 1)],
                                    start=(kc == 0 and not first),
                                    stop=(kc == 1))
                            if fine:
                                # last chunk: Scalar is idle (exp done)
                                # and Vector CASTs gate the a-pool
                                # rotation the tail matmuls wait on
                                nc.scalar.activation(
                                    yt[:, qq - 2 * qg,
                                       512 * do_:512 * (do_ + 1)],
                                    yp[:], Copy)
                            else:
                                nc.vector.tensor_copy(
                                    yt[:, qq - 2 * qg,
                                       512 * do_:512 * (do_ + 1)], yp[:])
                            if do_ == 1 and fine:
                                nc.sync.dma_start(
                                    y[b * T + 128 * q:
                                      b * T + 128 * (q + 1), :],
                                    yt[:, qq - 2 * qg, :])
                            elif do_ == 1 and qq == 2 * qg + 1:
                                q0 = 4 * j + 2 * qg
                                dst = y[b * T + 128 * q0:
                                        b * T + 128 * (q0 + 2),
                                        :].rearrange(
                                            "(blk p) c -> p blk c", p=128)
                                nc.sync.dma_start(dst, yt[:])
                        cl.append(f)
            return cl

        def emit_boot(xf):
            """Chunk 0's A-phase with the four q/k projection groups
            interleaved at the dc level (the q and k pairs live in the
            halves of two score-PSUM tiles, idle at boot), so PE
            progress tracks the x-block DMA feed instead of serializing
            group-by-group behind it."""
            sq = psum.tile([128, 1024], f32, tag="s", bufs=2)
            sk = psum.tile([128, 1024], f32, tag="s", bufs=2)
            # HAM warm-up: the PE clock boots throttled (K=4/8, 1.2GHz)
            # and un-throttles only after ~3.4us of sustained activity.
            # While the wq/x0 DMAs land, stream ~4us of zero-weight
            # matmuls into the sq accumulation group (0 contribution;
            # the real q-proj below joins the group with start=False)
            # ~26 x 512-col cold MMs ~ 11us: bridges the wq/x0 DMA
            # window (~12us) so HAM doesn't re-throttle before the
            # first real matmul
            for r in range(56):
                nc.tensor.matmul(
                    sq[:, 512 * (r % 2):512 * (r % 2 + 1)],
                    zt[:, 0:128], zt[:],
                    start=(r < 2), stop=False)
            for dc in range(8):
                for p in range(2):
                    nc.tensor.matmul(
                        sq[:, 512 * p:512 * (p + 1)],
                        wqf[:, dc, 128 * p:128 * (p + 1)],
                        xf[:, dc, :], start=False, stop=(dc == 7))
                    nc.tensor.matmul(
                        sk[:, 512 * p:512 * (p + 1)],
                        wkf[:, dc, 128 * p:128 * (p + 1)],
                        xf[:, dc, :], start=(dc == 0), stop=(dc == 7))
            for p in range(2):
                nc.vector.tensor_scalar_add(
                    qt_t[p][:, 0:512], sq[:, 512 * p:512 * (p + 1)],
                    bqk_t[:, p, 0:1])
                nc.vector.tensor_scalar_add(
                    kt_t[0][p][:, 0:512], sk[:, 512 * p:512 * (p + 1)],
                    bqk_t[:, p, 1:2])

            def ve(tt):
                def e(pv, dc):
                    nc.tensor.matmul(
                        pv[:, 0:VA], xf[:, dc, 128 * tt:128 * (tt + 1)],
                        wvf[:, dc, :], start=(dc == 0), stop=(dc == 7))
                return e

            def vfin(tt):
                def f(pv):
                    nc.vector.tensor_add(
                        va_t[0][tt][:], pv[:, 0:VA], bvb_t[:])
                return f

            for t0_ in (0, 2):
                pa = psum.tile([128, 512], f32, tag="a", bufs=2)
                pb = psum.tile([128, 512], f32, tag="a", bufs=2)
                for dc in range(8):
                    ve(t0_)(pa, dc)
                    ve(t0_ + 1)(pb, dc)
                vfin(t0_)(pa)
                vfin(t0_ + 1)(pb)

        def mix(a, b):
            out = []
            la, lb = list(a), list(b)
            while la or lb:
                if la:
                    out.append(la.pop(0))
                if lb:
                    out.append(lb.pop(0))
            return out

        # ---- pipelined schedule over 8 chunks (2 batches x 4 j) ----
        # Fill assignment per unit, balanced to each unit's absorbable
        # slack (big-j units run ACT-throttled without PE fill; unit 7
        # gets its own chunk's k/v projections, legal since those are
        # only read from score tile 4j on, plus two C sets):
        #   unit0: A(1) | unit1..4: C(ci-1)+A(ci+1) | unit5: A(6)
        #   unit6: C(4)+C(5)+A(7).q | unit7: A(7).kv+C(6) | end: C(7)
        bvb_t, wo_t = load_late_consts()
        emit_boot(xf0)
        xs = emit_loads(1)
        a_next = make_closures(1, xs)     # (q, kv) for chunk 1
        a7 = None
        csets = {}
        for ci in range(8):
            if ci + 2 <= 7:
                xs = emit_loads(ci + 2)
            if ci == 0:
                fill = a_next[0] + a_next[1]
            elif ci <= 4:
                # lead with A (projection) closures: the C(ci-1)
                # out-proj matmuls read ot, which waits on the previous
                # unit's normalize chain — putting one first parks it at
                # the PE FIFO head and stalls the whole unit behind it
                an = a_next[0] + a_next[1]
                fill = an[:4] + mix(csets.pop(ci - 1), an[4:])
            elif ci == 5:
                fill = a_next[0] + a_next[1]
            elif ci == 6:
                fill = mix(csets.pop(4) + csets.pop(5), a7[0])
            else:
                # kv first (deadline: AV reads va tiles 12-15 from the
                # hp0 flush, ~count 16), then two no-op slots to shift
                # C(6) into the back half where fill is scarce; hold 3
                # C(6) groups back for after the normalize section
                c6 = csets.pop(6)
                fill = a7[1] + [lambda: None] * 2 + c6[:3]
                held = c6[3:]
            emit_unit(ci, fill, split_norm=(ci == 7))
            if ci == 7:
                for f in held:
                    f()
            csets[ci] = make_phase_c(ci, fine=(ci == 7))
            if ci + 2 <= 7:
                nxt = make_closures(ci + 2, xs)
                if ci + 2 == 7:
                    a7 = nxt
                else:
                    a_next = nxt
        for f in csets.pop(7):
            f()

    nc.compile()
    return nc


def make_in_maps(inputs):
    import ml_dtypes
    bf = ml_dtypes.bfloat16
    x = inputs["x"].astype(np.float32)
    Wq, Wk, Wv, Wo = (inputs[k].astype(np.float32)
                      for k in ("Wq", "Wk", "Wv", "Wo"))
    bq, bk, bv = (inputs[k].astype(np.float32) for k in ("bq", "bk", "bv"))
    in_maps = []
    for c in range(8):
        dp, tp = divmod(c, TP)
        xt = np.ascontiguousarray(np.concatenate(
            [x[NB * dp + bb].T for bb in range(NB)], axis=0)).astype(bf)
        wq_c = np.ascontiguousarray(Wq[:, tp * QH:(tp + 1) * QH]).astype(bf)
        wk_c = np.ascontiguousarray(Wk[:, tp * QH:(tp + 1) * QH]).astype(bf)
        wv_c = np.zeros((D, VA), np.float32)
        bv_c = np.zeros((1, VA), np.float32)
        for hh in range(NH):
            g = tp * NH + hh
            wv_c[:, HV * hh:HV * hh + HD] = Wv[:, g * HD:(g + 1) * HD]
            bv_c[0, HV * hh:HV * hh + HD] = bv[g * HD:(g + 1) * HD]
            bv_c[0, HV * hh + HD] = 1.0
        wo_c = np.ascontiguousarray(Wo[tp * QH:(tp + 1) * QH, :]).astype(bf)
        bqk_c = np.ascontiguousarray(np.stack(
            [bq[tp * QH:(tp + 1) * QH], bk[tp * QH:(tp + 1) * QH]], axis=1))
        in_maps.append({
            "xt": xt, "wq": wq_c, "wk": wk_c, "wv": wv_c.astype(bf),
            "wo": wo_c, "bqk": bqk_c, "bv": bv_c.astype(bf),
            "tr": np.triu(np.ones((128, 128), np.float32)).astype(bf),
        })
    return in_maps


def kernel(**inputs):
    global LAST_EXEC_NS, _NC
    from concourse.bass_utils import run_bass_kernel_spmd

    if _NC is None:
        _NC = build_nc()
    in_maps = make_in_maps(inputs)
    res = run_bass_kernel_spmd(_NC, in_maps, core_ids=list(range(8)))

    bo = inputs["bo"].astype(np.float64)
    y_full = np.zeros((B, T, D), np.float64)
    for c in range(8):
        dp, tp = divmod(c, TP)
        yc = np.asarray(res.results[c]["y"]).astype(np.float64)
        for bb in range(NB):
            y_full[NB * dp + bb] += yc[bb * T:(bb + 1) * T, :]
    y_full += bo
    return y_full.astype(np.float32)

